# revision 17
# baseline (speedup 1.0000x reference)
"""Trainium2 Bass kernel for the two-branch sparse-attention fusion module.

Math (per batch b, tokens T = rgb/evt as (d=256, N=4096) d-major):
    s      = sum_n T[:, n]                           (256,)
    value[n] = T[:,n].v + c, v = (Wk^T Wq)^T s + N Wq^T bk, c = (Wk^T bq).s + N bq.bk
    w      = sigmoid((value_rgb - value_evt)/sqrt(d))
    out    = evt + w * (rgb - evt)

All-fp16 dataflow (fp32 only for DRAM I/O and PSUM accumulation):
    DMA (SWDGE): casting loads f32->fp16, casting stores fp16->f32
    ScalarE    : streaming row-sum partials (Copy+accum_out), sigmoid,
                 wb PSUM->SBUF fp16 copies, v_hi casts
    PE         : weight-product matvecs, fp16 value matmuls with v split
                 into hi+lo fp16 halves (error compensation), K=1 fp16
                 broadcast of w to 128 partitions
    DVE        : in-place fp16 blend A=(A-D), A*=wb, A+=D (2x mode)

Sharded data-parallel over batch: 8 cores x 2 batches, weights replicated.
"""

import numpy as np
from contextlib import ExitStack

import concourse.bass as bass
import concourse.tile as tile
from concourse import bacc, mybir
from concourse.bass import _add_dep_helper
from concourse.bass_utils import run_bass_kernel_spmd

F32 = mybir.dt.float32
FP16 = mybir.dt.float16

BS, DIM, HH, WW = 16, 256, 64, 64
N = HH * WW                 # 4096 tokens
NCORES = 8
BPC = BS // NCORES          # batches per core
PH = DIM // 128             # partition halves of the d dim
CH = 512                    # value-chunk (one PSUM bank of f32)
NCH = N // CH               # 8
LB = 2048                   # load block columns (1 MiB DRAM-side)
NLB = N // LB               # 2
RB = 2048                   # reduce block columns
NRB = N // RB               # 2
SB = 2048                   # store/blend block columns
NSB = N // SB               # 2
INV_SQRT_D = 1.0 / 16.0


def build_nc() -> bass.Bass:
    nc = bacc.Bacc()

    rgb = nc.declare_dram_parameter("rgb", [BPC, PH, 128, N], F32, isOutput=False)
    evt = nc.declare_dram_parameter("evt", [BPC, PH, 128, N], F32, isOutput=False)
    wts = {}
    for nm in ("Wq_a", "Wk_a", "Wq_d", "Wk_d"):
        wts[nm] = nc.declare_dram_parameter(nm, [PH, 128, DIM], F32, isOutput=False)
    bss = {}
    for nm in ("bq_a", "bk_a", "bq_d", "bk_d"):
        bss[nm] = nc.declare_dram_parameter(nm, [PH, 128, 1], F32, isOutput=False)
    out = nc.declare_dram_parameter("out", [BPC, PH, 128, N], F32, isOutput=True)

    with tile.TileContext(nc) as tc:
        _body(tc, rgb, evt, wts, bss, out)
    nc.finalize()
    return nc


def _precompute(tc, consts, ps_sm, W, B):
    """Weight products; the d branch carries a folded minus sign.
    PT and R are stored fp16 (they feed fp16 matvecs); U stays f32."""
    nc = tc.nc
    PT, U, R = {}, {}, {}
    for br, wq, wk, sign in (
        ("a", "Wq_a", "Wk_a", 1.0),
        ("d", "Wq_d", "Wk_d", -1.0),
    ):
        for jh in range(PH):
            ps = ps_sm.tile([128, DIM], F32, tag="ps_sm", name=f"psPT{br}{jh}")
            for oh in range(PH):
                nc.tensor.matmul(
                    ps,
                    lhsT=W[(wk, oh)][:, jh * 128 : (jh + 1) * 128],
                    rhs=W[(wq, oh)],
                    start=(oh == 0),
                    stop=(oh == PH - 1),
                )
            t = consts.tile([128, DIM], FP16, tag=f"PT{br}{jh}", name=f"PT{br}{jh}")
            nc.vector.tensor_scalar_mul(out=t, in0=ps, scalar1=sign)
            PT[(br, jh)] = t
        ps = ps_sm.tile([128, 2 * PH], F32, tag="ps_sm", name=f"psUR{br}")
        for ih in range(PH):
            for oh in range(PH):
                nc.tensor.matmul(
                    ps[:, ih : ih + 1],
                    lhsT=W[(wq, oh)][:, ih * 128 : (ih + 1) * 128],
                    rhs=B[("bk_" + br, oh)],
                    start=(oh == 0),
                    stop=(oh == PH - 1),
                )
        for jh in range(PH):
            for oh in range(PH):
                nc.tensor.matmul(
                    ps[:, PH + jh : PH + jh + 1],
                    lhsT=W[(wk, oh)][:, jh * 128 : (jh + 1) * 128],
                    rhs=B[("bq_" + br, oh)],
                    start=(oh == 0),
                    stop=(oh == PH - 1),
                )
        tU = consts.tile([128, PH], F32, tag=f"U{br}", name=f"U{br}")
        nc.vector.tensor_scalar_mul(out=tU, in0=ps[:, 0:PH], scalar1=float(sign * N))
        tR = consts.tile([128, PH], FP16, tag=f"R{br}", name=f"R{br}")
        nc.vector.tensor_scalar_mul(out=tR, in0=ps[:, PH : 2 * PH], scalar1=sign)
        U[("full", br)] = tU
        for ih in range(PH):
            U[(br, ih)] = tU[:, ih : ih + 1]
        for jh in range(PH):
            R[(br, jh)] = tR[:, jh : jh + 1]

    # batch-independent bias-dot part of c_diff: N*(bq_a.bk_a - bq_d.bk_d)
    ps = ps_sm.tile([1, 1], F32, tag="ps_sm", name="psCb")
    k = 0
    for bq, bk, sgn in (("bq_a", "bk_a", 1), ("bq_d", "bk_d", -1)):
        for oh in range(PH):
            t = consts.tile([128, 1], F32, tag=f"bkN{bk}{oh}", name=f"bkN{bk}{oh}")
            nc.vector.tensor_scalar_mul(out=t, in0=B[(bk, oh)], scalar1=float(sgn * N))
            nc.tensor.matmul(ps, lhsT=B[(bq, oh)], rhs=t, start=(k == 0), stop=(k == 3))
            k += 1
    c_bias = consts.tile([1, 1], FP16, tag="c_bias")
    nc.vector.tensor_scalar_mul(out=c_bias, in0=ps, scalar1=1.0)
    return PT, U, R, c_bias


def _body(tc, rgb, evt, wts, bss, out):
    nc = tc.nc
    ACT = mybir.ActivationFunctionType
    with ExitStack() as ctx:
        consts = ctx.enter_context(tc.tile_pool(name="consts", bufs=1))
        data = ctx.enter_context(tc.tile_pool(name="data", bufs=2))
        wbp = ctx.enter_context(tc.tile_pool(name="wbp", bufs=2))
        small = ctx.enter_context(tc.tile_pool(name="small", bufs=2))
        wchunk = ctx.enter_context(tc.tile_pool(name="wchunk", bufs=4))
        ps_val = ctx.enter_context(tc.tile_pool(name="ps_val", bufs=3, space="PSUM"))
        ps_wb = ctx.enter_context(tc.tile_pool(name="ps_wb", bufs=3, space="PSUM"))
        ps_sm = ctx.enter_context(tc.tile_pool(name="ps_sm", bufs=1, space="PSUM"))

        # ---- load weights + biases (plain f32 HWDGE) -------------------
        W = {}
        for nm in ("Wq_a", "Wk_a", "Wq_d", "Wk_d"):
            for h in range(PH):
                t = consts.tile([128, DIM], F32, tag=f"{nm}{h}", name=f"{nm}{h}")
                nc.sync.dma_start(out=t, in_=wts[nm][h])
                W[(nm, h)] = t
        B = {}
        for nm in ("bq_a", "bk_a", "bq_d", "bk_d"):
            for h in range(PH):
                t = consts.tile([128, 1], F32, tag=f"{nm}{h}", name=f"b{nm}{h}")
                nc.sync.dma_start(out=t, in_=bss[nm][h])
                B[(nm, h)] = t

        ones_row = consts.tile([1, 128], FP16, tag="ones")
        nc.vector.memset(ones_row, 1.0)
        one_one = consts.tile([1, 1], FP16, tag="one_one")
        nc.vector.memset(one_one, 1.0)
        garbage = consts.tile([128, 1], F32, tag="garbage")

        PT, U, R, c_bias = _precompute(tc, consts, ps_sm, W, B)

        st = [dict() for _ in range(BPC)]

        def emit_loads(b, after=None):
            # casting SWDGE loads: DRAM f32 -> SBUF fp16. `after` orders this
            # batch's stream behind the previous batch's last load so early
            # DMAs complete early (completions smear across co-resident DMAs).
            A, Dv = {}, {}
            first = last = None
            for h in range(PH):
                A[h] = data.tile([128, N], FP16, tag=f"A{h}", name=f"A{h}_{b}")
                Dv[h] = data.tile([128, N], FP16, tag=f"D{h}", name=f"D{h}_{b}")
                for blk in range(NLB):
                    sl = slice(blk * LB, (blk + 1) * LB)
                    i1 = nc.gpsimd.dma_start(out=A[h][:, sl], in_=rgb[b, h][:, sl])
                    i2 = nc.gpsimd.dma_start(out=Dv[h][:, sl], in_=evt[b, h][:, sl])
                    if first is None:
                        first = i1
                    last = i2
            if after is not None:
                _add_dep_helper(
                    first.ins, after.ins, sync=True,
                    reason="batch loads ordered to avoid completion smearing",
                )
            st[b] = dict(A=A, Dv=Dv, last_load=last)

        def make_red_ops(b):
            # row-sum partial ops on ScalarE (Copy + accum_out, discard out)
            A, Dv = st[b]["A"], st[b]["Dv"]
            S4 = {}
            ops = []
            for key, tiles in (("a", A), ("d", Dv)):
                for h in range(PH):
                    s4 = small.tile(
                        [128, NRB], FP16, tag=f"s4{key}{h}", name=f"s4{key}{h}_{b}"
                    )
                    S4[(key, h)] = s4

                    def red(t, s, dst):
                        with nc.allow_low_precision(
                            reason="fp16 write of f32-accumulated partial"
                        ):
                            nc.vector.reduce_sum(
                                out=dst, in_=t[:, s], axis=mybir.AxisListType.X
                            )

                    for rb in range(NRB):
                        sl = slice(rb * RB, (rb + 1) * RB)
                        ops.append(
                            lambda t=tiles[h], s=sl, dst=s4[:, rb : rb + 1]: red(
                                t, s, dst
                            )
                        )
            st[b]["S4"] = S4
            return ops

        def stage2(b, interleave_ops=()):
            A, Dv, S4 = st[b]["A"], st[b]["Dv"], st[b]["S4"]

            # c_diff = sum_j r[j] s[j] (both branches) + c_bias -- first so the
            # sigmoid bias is ready before the value chunks complete
            ps_c = ps_sm.tile([1, 1], F32, tag="ps_sm", name=f"psc_{b}")
            terms = [
                (S4[(br, jh)][:, rb : rb + 1], R[(br, jh)])
                for br in ("a", "d")
                for jh in range(PH)
                for rb in range(NRB)
            ]
            for i, (l, r) in enumerate(terms):
                nc.tensor.matmul(ps_c, lhsT=l, rhs=r, start=(i == 0), stop=False)
            nc.tensor.matmul(ps_c, lhsT=c_bias, rhs=one_one, start=False, stop=True)
            c16 = small.tile([1, 1], F32, tag="c16", name=f"c16_{b}")
            nc.scalar.mul(out=c16, in_=ps_c, mul=INV_SQRT_D)

            # v = PT @ s + U per branch (s consumed as NRB fp16 partials),
            # one (128, PH) psum group per branch, then split into fp16 hi+lo
            VH, VL = {}, {}
            for br in ("a", "d"):
                ps = ps_sm.tile([128, PH], F32, tag="ps_sm", name=f"psv{br}_{b}")
                for ih in range(PH):
                    k = 0
                    for jh in range(PH):
                        for rb in range(NRB):
                            nc.tensor.matmul(
                                ps[:, ih : ih + 1],
                                lhsT=PT[(br, jh)][:, ih * 128 : (ih + 1) * 128],
                                rhs=S4[(br, jh)][:, rb : rb + 1],
                                start=(k == 0),
                                stop=(k == PH * NRB - 1),
                            )
                            k += 1
                v = small.tile([128, PH], F32, tag=f"v{br}", name=f"v{br}_{b}")
                nc.vector.tensor_add(out=v, in0=ps, in1=U[("full", br)])
                vh = small.tile([128, PH], FP16, tag=f"vh{br}", name=f"vh{br}_{b}")
                nc.vector.tensor_scalar_mul(out=vh, in0=v, scalar1=1.0)
                vl = small.tile([128, PH], FP16, tag=f"vl{br}", name=f"vl{br}_{b}")
                nc.vector.tensor_sub(out=vl, in0=v, in1=vh)
                for ih in range(PH):
                    VH[(br, ih)] = vh[:, ih : ih + 1]
                    VL[(br, ih)] = vl[:, ih : ih + 1]

            # value matmuls, lhsT-major over 4-chunk groups (LDWEIGHTS reuse)
            mms = [
                (VH[("a", 0)], A[0]), (VL[("a", 0)], A[0]),
                (VH[("a", 1)], A[1]), (VL[("a", 1)], A[1]),
                (VH[("d", 0)], Dv[0]), (VL[("d", 0)], Dv[0]),
                (VH[("d", 1)], Dv[1]), (VL[("d", 1)], Dv[1]),
            ]
            wb_sb = wbp.tile([128, N], FP16, tag="wb_sb", name=f"wb_sb_{b}")
            il = list(interleave_ops)

            def emit_tail_w(ich):
                wb = ps_wb.tile([128, CH], F32, tag="wb", name=f"wb{ich}_{b}")
                nc.tensor.matmul(
                    wb, lhsT=ones_row, rhs=wrows[ich], start=True, stop=True
                )
                nc.scalar.copy(out=wb_sb[:, ich * CH : (ich + 1) * CH], in_=wb)

            wrows = [None] * NCH
            for ich in range(NCH):
                sl = slice(ich * CH, (ich + 1) * CH)
                psv = ps_val.tile([1, CH], F32, tag="psv", name=f"psval{ich}_{b}")
                for i, (v, t) in enumerate(mms):
                    nc.tensor.matmul(
                        psv, lhsT=v, rhs=t[:, sl],
                        start=(i == 0), stop=(i == len(mms) - 1),
                    )
                if il:
                    il.pop(0)()
                wrow = wchunk.tile([1, CH], FP16, tag="wrow", name=f"wrow{ich}_{b}")
                nc.scalar.activation(
                    out=wrow, in_=psv,
                    func=ACT.Sigmoid, bias=c16, scale=INV_SQRT_D,
                )
                wrows[ich] = wrow
                if ich >= 1:
                    emit_tail_w(ich - 1)
            emit_tail_w(NCH - 1)
            for op in il:
                op()

            # in-place fp16 blend on A (all 2x DVE mode), casting stores
            for h in range(PH):
                for sb in range(NSB):
                    sl = slice(sb * SB, (sb + 1) * SB)
                    nc.vector.tensor_sub(out=A[h][:, sl], in0=A[h][:, sl], in1=Dv[h][:, sl])
            for h in range(PH):
                for sb in range(NSB):
                    sl = slice(sb * SB, (sb + 1) * SB)
                    nc.vector.tensor_mul(out=A[h][:, sl], in0=A[h][:, sl], in1=wb_sb[:, sl])
            for h in range(PH):
                for sb in range(NSB):
                    sl = slice(sb * SB, (sb + 1) * SB)
                    nc.vector.tensor_add(out=A[h][:, sl], in0=A[h][:, sl], in1=Dv[h][:, sl])
                    nc.gpsimd.dma_start(out=out[b, h][:, sl], in_=A[h][:, sl])

        # ---- emission schedule ----------------------------------------
        emit_loads(0)
        red0 = make_red_ops(0)
        for op in red0:
            op()
        emit_loads(1, after=st[0]["last_load"])
        red1 = make_red_ops(1)
        stage2(0, interleave_ops=red1)
        stage2(1)


_NC_CACHE = None


def _get_nc():
    global _NC_CACHE
    if _NC_CACHE is None:
        _NC_CACHE = build_nc()
    return _NC_CACHE


def _make_in_maps(inputs):
    rgb = np.ascontiguousarray(np.asarray(inputs["rgb"], dtype=np.float32)).reshape(
        BS, PH, 128, N
    )
    evt = np.ascontiguousarray(np.asarray(inputs["evt"], dtype=np.float32)).reshape(
        BS, PH, 128, N
    )
    base = {}
    for nm in ("Wq_a", "Wk_a", "Wq_d", "Wk_d"):
        base[nm] = np.ascontiguousarray(
            np.asarray(inputs[nm], dtype=np.float32)
        ).reshape(PH, 128, DIM)
    for nm in ("bq_a", "bk_a", "bq_d", "bk_d"):
        base[nm] = np.ascontiguousarray(
            np.asarray(inputs[nm], dtype=np.float32)
        ).reshape(PH, 128, 1)
    in_maps = []
    for c in range(NCORES):
        m = dict(base)
        m["rgb"] = np.ascontiguousarray(rgb[c * BPC : (c + 1) * BPC])
        m["evt"] = np.ascontiguousarray(evt[c * BPC : (c + 1) * BPC])
        in_maps.append(m)
    return in_maps


def run(inputs, trace=False):
    nc = _get_nc()
    in_maps = _make_in_maps(inputs)
    res = run_bass_kernel_spmd(nc, in_maps, core_ids=list(range(NCORES)), trace=trace)
    outs = [
        np.asarray(res.results[i]["out"]).reshape(BPC, DIM, HH, WW)
        for i in range(NCORES)
    ]
    full = np.concatenate(outs, axis=0)
    return full, res


def kernel(**inputs) -> np.ndarray:
    full, _ = run(inputs, trace=False)
    return full


# revision 18
# speedup vs baseline: 1.2614x; 1.2614x over previous
"""Trainium2 Bass kernel for the two-branch sparse-attention fusion module.

Math (per batch b, tokens T = rgb/evt as (d=256, N=4096) d-major):
    s      = sum_n T[:, n]                           (256,)
    value[n] = T[:,n].v + c, v = (Wk^T Wq)^T s + N Wq^T bk, c = (Wk^T bq).s + N bq.bk
    w      = sigmoid((value_rgb - value_evt)/sqrt(d))
    out    = evt + w * (rgb - evt)

All-fp16 dataflow (fp32 only for DRAM I/O and PSUM accumulation):
    DMA (SWDGE): casting loads f32->fp16, casting stores fp16->f32
    ScalarE    : streaming row-sum partials (Copy+accum_out), sigmoid,
                 wb PSUM->SBUF fp16 copies, v_hi casts
    PE         : weight-product matvecs, fp16 value matmuls with v split
                 into hi+lo fp16 halves (error compensation), K=1 fp16
                 broadcast of w to 128 partitions
    DVE        : in-place fp16 blend A=(A-D), A*=wb, A+=D (2x mode)

Sharded data-parallel over batch: 8 cores x 2 batches, weights replicated.
"""

import numpy as np
from contextlib import ExitStack

import concourse.bass as bass
import concourse.tile as tile
from concourse import bacc, mybir
from concourse.bass import _add_dep_helper
from concourse.bass_utils import run_bass_kernel_spmd

F32 = mybir.dt.float32
FP16 = mybir.dt.float16

BS, DIM, HH, WW = 16, 256, 64, 64
N = HH * WW                 # 4096 tokens
NCORES = 8
BPC = BS // NCORES          # batches per core
PH = DIM // 128             # partition halves of the d dim
CH = 512                    # value-chunk (one PSUM bank of f32)
NCH = N // CH               # 8
LB = 2048                   # load block columns (1 MiB DRAM-side)
NLB = N // LB               # 2
RB = 2048                   # reduce block columns
NRB = N // RB               # 2
SB = 2048                   # store/blend block columns
NSB = N // SB               # 2
INV_SQRT_D = 1.0 / 16.0


def build_nc() -> bass.Bass:
    nc = bacc.Bacc()

    rgb = nc.declare_dram_parameter("rgb", [BPC, PH, 128, N], F32, isOutput=False)
    evt = nc.declare_dram_parameter("evt", [BPC, PH, 128, N], F32, isOutput=False)
    wts = {}
    for nm in ("Wq_a", "Wk_a", "Wq_d", "Wk_d"):
        wts[nm] = nc.declare_dram_parameter(nm, [PH, 128, DIM], F32, isOutput=False)
    bss = {}
    for nm in ("bq_a", "bk_a", "bq_d", "bk_d"):
        bss[nm] = nc.declare_dram_parameter(nm, [PH, 128, 1], F32, isOutput=False)
    out = nc.declare_dram_parameter("out", [BPC, PH, 128, N], F32, isOutput=True)

    with tile.TileContext(nc) as tc:
        _body(tc, rgb, evt, wts, bss, out)
    nc.finalize()
    return nc


def _precompute(tc, consts, ps_sm, W, B):
    """Weight products; the d branch carries a folded minus sign.
    PT and R are stored fp16 (they feed fp16 matvecs); U stays f32."""
    nc = tc.nc
    PT, U, R = {}, {}, {}
    for br, wq, wk, sign in (
        ("a", "Wq_a", "Wk_a", 1.0),
        ("d", "Wq_d", "Wk_d", -1.0),
    ):
        for jh in range(PH):
            ps = ps_sm.tile([128, DIM], F32, tag="ps_sm", name=f"psPT{br}{jh}")
            for oh in range(PH):
                nc.tensor.matmul(
                    ps,
                    lhsT=W[(wk, oh)][:, jh * 128 : (jh + 1) * 128],
                    rhs=W[(wq, oh)],
                    start=(oh == 0),
                    stop=(oh == PH - 1),
                )
            t = consts.tile([128, DIM], FP16, tag=f"PT{br}{jh}", name=f"PT{br}{jh}")
            nc.vector.tensor_scalar_mul(out=t, in0=ps, scalar1=sign)
            PT[(br, jh)] = t
        ps = ps_sm.tile([128, 2 * PH], F32, tag="ps_sm", name=f"psUR{br}")
        for ih in range(PH):
            for oh in range(PH):
                nc.tensor.matmul(
                    ps[:, ih : ih + 1],
                    lhsT=W[(wq, oh)][:, ih * 128 : (ih + 1) * 128],
                    rhs=B[("bk_" + br, oh)],
                    start=(oh == 0),
                    stop=(oh == PH - 1),
                )
        for jh in range(PH):
            for oh in range(PH):
                nc.tensor.matmul(
                    ps[:, PH + jh : PH + jh + 1],
                    lhsT=W[(wk, oh)][:, jh * 128 : (jh + 1) * 128],
                    rhs=B[("bq_" + br, oh)],
                    start=(oh == 0),
                    stop=(oh == PH - 1),
                )
        tU = consts.tile([128, PH], F32, tag=f"U{br}", name=f"U{br}")
        nc.vector.tensor_scalar_mul(out=tU, in0=ps[:, 0:PH], scalar1=float(sign * N))
        tR = consts.tile([128, PH], FP16, tag=f"R{br}", name=f"R{br}")
        nc.vector.tensor_scalar_mul(out=tR, in0=ps[:, PH : 2 * PH], scalar1=sign)
        U[("full", br)] = tU
        for ih in range(PH):
            U[(br, ih)] = tU[:, ih : ih + 1]
        for jh in range(PH):
            R[(br, jh)] = tR[:, jh : jh + 1]

    # batch-independent bias-dot part of c_diff: N*(bq_a.bk_a - bq_d.bk_d)
    ps = ps_sm.tile([1, 1], F32, tag="ps_sm", name="psCb")
    k = 0
    for bq, bk, sgn in (("bq_a", "bk_a", 1), ("bq_d", "bk_d", -1)):
        for oh in range(PH):
            t = consts.tile([128, 1], F32, tag=f"bkN{bk}{oh}", name=f"bkN{bk}{oh}")
            nc.vector.tensor_scalar_mul(out=t, in0=B[(bk, oh)], scalar1=float(sgn * N))
            nc.tensor.matmul(ps, lhsT=B[(bq, oh)], rhs=t, start=(k == 0), stop=(k == 3))
            k += 1
    c_bias = consts.tile([1, 1], FP16, tag="c_bias")
    nc.vector.tensor_scalar_mul(out=c_bias, in0=ps, scalar1=1.0)
    return PT, U, R, c_bias


def _body(tc, rgb, evt, wts, bss, out):
    nc = tc.nc
    ACT = mybir.ActivationFunctionType
    with ExitStack() as ctx:
        consts = ctx.enter_context(tc.tile_pool(name="consts", bufs=1))
        data = ctx.enter_context(tc.tile_pool(name="data", bufs=2))
        mpool = ctx.enter_context(tc.tile_pool(name="mpool", bufs=2))
        wbp = ctx.enter_context(tc.tile_pool(name="wbp", bufs=2))
        small = ctx.enter_context(tc.tile_pool(name="small", bufs=2))
        wchunk = ctx.enter_context(tc.tile_pool(name="wchunk", bufs=4))
        ps_val = ctx.enter_context(tc.tile_pool(name="ps_val", bufs=5, space="PSUM"))
        ps_wb = ctx.enter_context(tc.tile_pool(name="ps_wb", bufs=3, space="PSUM"))
        ps_sm = ctx.enter_context(tc.tile_pool(name="ps_sm", bufs=1, space="PSUM"))

        # ---- load weights + biases (plain f32 HWDGE) -------------------
        W = {}
        for nm in ("Wq_a", "Wk_a", "Wq_d", "Wk_d"):
            for h in range(PH):
                t = consts.tile([128, DIM], F32, tag=f"{nm}{h}", name=f"{nm}{h}")
                nc.sync.dma_start(out=t, in_=wts[nm][h])
                W[(nm, h)] = t
        B = {}
        for nm in ("bq_a", "bk_a", "bq_d", "bk_d"):
            for h in range(PH):
                t = consts.tile([128, 1], F32, tag=f"{nm}{h}", name=f"b{nm}{h}")
                nc.sync.dma_start(out=t, in_=bss[nm][h])
                B[(nm, h)] = t

        ones_row = consts.tile([1, 128], FP16, tag="ones")
        nc.vector.memset(ones_row, 1.0)
        one_one = consts.tile([1, 1], FP16, tag="one_one")
        nc.vector.memset(one_one, 1.0)
        garbage = consts.tile([128, 1], F32, tag="garbage")

        PT, U, R, c_bias = _precompute(tc, consts, ps_sm, W, B)

        st = [dict() for _ in range(BPC)]

        def emit_loads(b, after=None):
            # casting SWDGE loads: DRAM f32 -> SBUF fp16. `after` orders this
            # batch's stream behind the previous batch's last load so early
            # DMAs complete early (completions smear across co-resident DMAs).
            A, Dv = {}, {}
            first = last = None
            for h in range(PH):
                A[h] = data.tile([128, N], FP16, tag=f"A{h}", name=f"A{h}_{b}")
                Dv[h] = data.tile([128, N], FP16, tag=f"D{h}", name=f"D{h}_{b}")
                for blk in range(NLB):
                    sl = slice(blk * LB, (blk + 1) * LB)
                    i1 = nc.gpsimd.dma_start(out=A[h][:, sl], in_=rgb[b, h][:, sl])
                    i2 = nc.gpsimd.dma_start(out=Dv[h][:, sl], in_=evt[b, h][:, sl])
                    if first is None:
                        first = i1
                    last = i2
            if after is not None:
                _add_dep_helper(
                    first.ins, after.ins, sync=True,
                    reason="batch loads ordered to avoid completion smearing",
                )
            st[b] = dict(A=A, Dv=Dv, last_load=last)

        def emit_reds(b):
            # a-branch row-sum partials on ScalarE (Copy + accum, discard out)
            A = st[b]["A"]
            SA = {}
            for h in range(PH):
                s4 = small.tile([128, NRB], FP16, tag=f"s4a{h}", name=f"s4a{h}_{b}")
                SA[h] = s4
                for rb in range(NRB):
                    sl = slice(rb * RB, (rb + 1) * RB)
                    with nc.allow_low_precision(
                        reason="fp16 write of f32-accumulated partial"
                    ):
                        nc.scalar.activation(
                            out=garbage.broadcast_to([128, RB]),
                            in_=A[h][:, sl],
                            func=ACT.Copy,
                            accum_out=s4[:, rb : rb + 1],
                        )
            st[b]["SA"] = SA

        def emit_subs(b):
            # M = A - D on DVE (fp16 2x), accumulating sm = rowsum(A - D)
            A, Dv = st[b]["A"], st[b]["Dv"]
            M, SM = {}, {}
            for h in range(PH):
                M[h] = mpool.tile([128, N], FP16, tag=f"M{h}", name=f"M{h}_{b}")
                sm4 = small.tile([128, NRB], F32, tag=f"sm4{h}", name=f"sm4{h}_{b}")
                SM[h] = sm4
                for rb in range(NRB):
                    sl = slice(rb * RB, (rb + 1) * RB)
                    nc.vector.scalar_tensor_tensor(
                        out=M[h][:, sl],
                        in0=A[h][:, sl],
                        scalar=1.0,
                        in1=Dv[h][:, sl],
                        op0=mybir.AluOpType.mult,
                        op1=mybir.AluOpType.subtract,
                        accum_out=sm4[:, rb : rb + 1],
                    )
            st[b]["M"] = M
            st[b]["SM"] = SM

        def stage2(b):
            A, Dv, M = st[b]["A"], st[b]["Dv"], st[b]["M"]
            SA, SM = st[b]["SA"], st[b]["SM"]

            # d-branch partials: s_d = s_a - sm (tiny)
            S4 = {}
            for h in range(PH):
                S4[("a", h)] = SA[h]
                sd = small.tile([128, NRB], FP16, tag=f"s4d{h}", name=f"s4d{h}_{b}")
                with nc.allow_low_precision(reason="tiny fp16 partial"):
                    nc.vector.tensor_sub(out=sd, in0=SA[h], in1=SM[h])
                S4[("d", h)] = sd

            # c_diff = sum_j r[j] s[j] (both branches) + c_bias
            ps_c = ps_sm.tile([1, 1], F32, tag="ps_sm", name=f"psc_{b}")
            terms = [
                (S4[(br, jh)][:, rb : rb + 1], R[(br, jh)])
                for br in ("a", "d")
                for jh in range(PH)
                for rb in range(NRB)
            ]
            for i, (l, r) in enumerate(terms):
                nc.tensor.matmul(ps_c, lhsT=l, rhs=r, start=(i == 0), stop=False)
            nc.tensor.matmul(ps_c, lhsT=c_bias, rhs=one_one, start=False, stop=True)
            c16 = small.tile([1, 1], F32, tag="c16", name=f"c16_{b}")
            nc.scalar.mul(out=c16, in_=ps_c, mul=INV_SQRT_D)

            # v = PT @ s + U per branch, split into fp16 hi+lo
            VH, VL = {}, {}
            for br in ("a", "d"):
                ps = ps_sm.tile([128, PH], F32, tag="ps_sm", name=f"psv{br}_{b}")
                for ih in range(PH):
                    k = 0
                    for jh in range(PH):
                        for rb in range(NRB):
                            nc.tensor.matmul(
                                ps[:, ih : ih + 1],
                                lhsT=PT[(br, jh)][:, ih * 128 : (ih + 1) * 128],
                                rhs=S4[(br, jh)][:, rb : rb + 1],
                                start=(k == 0),
                                stop=(k == PH * NRB - 1),
                            )
                            k += 1
                v = small.tile([128, PH], F32, tag=f"v{br}", name=f"v{br}_{b}")
                nc.vector.tensor_add(out=v, in0=ps, in1=U[("full", br)])
                vh = small.tile([128, PH], FP16, tag=f"vh{br}", name=f"vh{br}_{b}")
                nc.vector.tensor_scalar_mul(out=vh, in0=v, scalar1=1.0)
                vl = small.tile([128, PH], FP16, tag=f"vl{br}", name=f"vl{br}_{b}")
                nc.vector.tensor_sub(out=vl, in0=v, in1=vh)
                for ih in range(PH):
                    VH[(br, ih)] = vh[:, ih : ih + 1]
                    VL[(br, ih)] = vl[:, ih : ih + 1]

            # value chunks: 8 fp16 matmuls -> sigmoid -> partition_broadcast
            mms = [
                (VH[("a", 0)], A[0]), (VL[("a", 0)], A[0]),
                (VH[("a", 1)], A[1]), (VL[("a", 1)], A[1]),
                (VH[("d", 0)], Dv[0]), (VL[("d", 0)], Dv[0]),
                (VH[("d", 1)], Dv[1]), (VL[("d", 1)], Dv[1]),
            ]
            wb_sb = wbp.tile([128, N], FP16, tag="wb_sb", name=f"wb_sb_{b}")
            for ich in range(NCH):
                sl = slice(ich * CH, (ich + 1) * CH)
                psv = ps_val.tile([1, CH], F32, tag="psv", name=f"psval{ich}_{b}")
                for i, (v, t) in enumerate(mms):
                    nc.tensor.matmul(
                        psv, lhsT=v, rhs=t[:, sl],
                        start=(i == 0), stop=(i == len(mms) - 1),
                    )
                wrow = wchunk.tile([1, CH], FP16, tag="wrow", name=f"wrow{ich}_{b}")
                nc.scalar.activation(
                    out=wrow, in_=psv,
                    func=ACT.Sigmoid, bias=c16, scale=INV_SQRT_D,
                )
                nc.gpsimd.partition_broadcast(wb_sb[:, sl], wrow)

            # blend: M *= wb (2x), A = M + D (in-place A), casting stores
            for h in range(PH):
                for sb in range(NSB):
                    sl = slice(sb * SB, (sb + 1) * SB)
                    nc.vector.tensor_mul(out=M[h][:, sl], in0=M[h][:, sl], in1=wb_sb[:, sl])
            for h in range(PH):
                for sb in range(NSB):
                    sl = slice(sb * SB, (sb + 1) * SB)
                    nc.vector.tensor_add(out=A[h][:, sl], in0=M[h][:, sl], in1=Dv[h][:, sl])
                    nc.gpsimd.dma_start(out=out[b, h][:, sl], in_=A[h][:, sl])

        # ---- emission schedule ----------------------------------------
        emit_loads(0)
        emit_reds(0)
        emit_subs(0)
        emit_loads(1, after=st[0]["last_load"])
        emit_reds(1)
        emit_subs(1)
        stage2(0)
        stage2(1)


_NC_CACHE = None


def _get_nc():
    global _NC_CACHE
    if _NC_CACHE is None:
        _NC_CACHE = build_nc()
    return _NC_CACHE


def _make_in_maps(inputs):
    rgb = np.ascontiguousarray(np.asarray(inputs["rgb"], dtype=np.float32)).reshape(
        BS, PH, 128, N
    )
    evt = np.ascontiguousarray(np.asarray(inputs["evt"], dtype=np.float32)).reshape(
        BS, PH, 128, N
    )
    base = {}
    for nm in ("Wq_a", "Wk_a", "Wq_d", "Wk_d"):
        base[nm] = np.ascontiguousarray(
            np.asarray(inputs[nm], dtype=np.float32)
        ).reshape(PH, 128, DIM)
    for nm in ("bq_a", "bk_a", "bq_d", "bk_d"):
        base[nm] = np.ascontiguousarray(
            np.asarray(inputs[nm], dtype=np.float32)
        ).reshape(PH, 128, 1)
    in_maps = []
    for c in range(NCORES):
        m = dict(base)
        m["rgb"] = np.ascontiguousarray(rgb[c * BPC : (c + 1) * BPC])
        m["evt"] = np.ascontiguousarray(evt[c * BPC : (c + 1) * BPC])
        in_maps.append(m)
    return in_maps


def run(inputs, trace=False):
    nc = _get_nc()
    in_maps = _make_in_maps(inputs)
    res = run_bass_kernel_spmd(nc, in_maps, core_ids=list(range(NCORES)), trace=trace)
    outs = [
        np.asarray(res.results[i]["out"]).reshape(BPC, DIM, HH, WW)
        for i in range(NCORES)
    ]
    full = np.concatenate(outs, axis=0)
    return full, res


def kernel(**inputs) -> np.ndarray:
    full, _ = run(inputs, trace=False)
    return full


# revision 19
# speedup vs baseline: 1.3304x; 1.0547x over previous
"""Trainium2 Bass kernel for the two-branch sparse-attention fusion module.

Math (per batch b, tokens T = rgb/evt as (d=256, N=4096) d-major):
    s      = sum_n T[:, n]                           (256,)
    value[n] = T[:,n].v + c, v = (Wk^T Wq)^T s + N Wq^T bk, c = (Wk^T bq).s + N bq.bk
    w      = sigmoid((value_rgb - value_evt)/sqrt(d))
    out    = evt + w * (rgb - evt)

All-fp16 dataflow (fp32 only for DRAM I/O and PSUM accumulation):
    DMA (SWDGE): casting loads f32->fp16, casting stores fp16->f32
    ScalarE    : streaming row-sum partials (Copy+accum_out), sigmoid,
                 wb PSUM->SBUF fp16 copies, v_hi casts
    PE         : weight-product matvecs, fp16 value matmuls with v split
                 into hi+lo fp16 halves (error compensation), K=1 fp16
                 broadcast of w to 128 partitions
    DVE        : in-place fp16 blend A=(A-D), A*=wb, A+=D (2x mode)

Sharded data-parallel over batch: 8 cores x 2 batches, weights replicated.
"""

import numpy as np
from contextlib import ExitStack

import concourse.bass as bass
import concourse.tile as tile
from concourse import bacc, mybir
from concourse.bass import _add_dep_helper
from concourse.bass_utils import run_bass_kernel_spmd

F32 = mybir.dt.float32
FP16 = mybir.dt.float16

BS, DIM, HH, WW = 16, 256, 64, 64
N = HH * WW                 # 4096 tokens
NCORES = 8
BPC = BS // NCORES          # batches per core
PH = DIM // 128             # partition halves of the d dim
CH = 512                    # value-chunk (one PSUM bank of f32)
NCH = N // CH               # 8
LB = 2048                   # load block columns (1 MiB DRAM-side)
NLB = N // LB               # 2
RB = 2048                   # reduce block columns
NRB = N // RB               # 2
SB = 2048                   # store/blend block columns
NSB = N // SB               # 2
INV_SQRT_D = 1.0 / 16.0


def build_nc() -> bass.Bass:
    nc = bacc.Bacc()

    rgb = nc.declare_dram_parameter("rgb", [BPC, PH, 128, N], F32, isOutput=False)
    evt = nc.declare_dram_parameter("evt", [BPC, PH, 128, N], F32, isOutput=False)
    wts = {}
    for nm in ("Wq_a", "Wk_a", "Wq_d", "Wk_d"):
        wts[nm] = nc.declare_dram_parameter(nm, [PH, 128, DIM], F32, isOutput=False)
    bss = {}
    for nm in ("bq_a", "bk_a", "bq_d", "bk_d"):
        bss[nm] = nc.declare_dram_parameter(nm, [PH, 128, 1], F32, isOutput=False)
    out = nc.declare_dram_parameter("out", [BPC, PH, 128, N], F32, isOutput=True)

    with tile.TileContext(nc) as tc:
        _body(tc, rgb, evt, wts, bss, out)
    nc.finalize()
    return nc


def _precompute(tc, consts, ps_sm, W, B):
    """Weight products; the d branch carries a folded minus sign.
    PT and R are stored fp16 (they feed fp16 matvecs); U stays f32."""
    nc = tc.nc
    PT, U, R = {}, {}, {}
    for br, wq, wk, sign in (
        ("a", "Wq_a", "Wk_a", 1.0),
        ("d", "Wq_d", "Wk_d", -1.0),
    ):
        for jh in range(PH):
            ps = ps_sm.tile([128, DIM], F32, tag="ps_sm", name=f"psPT{br}{jh}")
            for oh in range(PH):
                nc.tensor.matmul(
                    ps,
                    lhsT=W[(wk, oh)][:, jh * 128 : (jh + 1) * 128],
                    rhs=W[(wq, oh)],
                    start=(oh == 0),
                    stop=(oh == PH - 1),
                )
            t = consts.tile([128, DIM], FP16, tag=f"PT{br}{jh}", name=f"PT{br}{jh}")
            nc.vector.tensor_scalar_mul(out=t, in0=ps, scalar1=sign)
            PT[(br, jh)] = t
        ps = ps_sm.tile([128, 2 * PH], F32, tag="ps_sm", name=f"psUR{br}")
        for ih in range(PH):
            for oh in range(PH):
                nc.tensor.matmul(
                    ps[:, ih : ih + 1],
                    lhsT=W[(wq, oh)][:, ih * 128 : (ih + 1) * 128],
                    rhs=B[("bk_" + br, oh)],
                    start=(oh == 0),
                    stop=(oh == PH - 1),
                )
        for jh in range(PH):
            for oh in range(PH):
                nc.tensor.matmul(
                    ps[:, PH + jh : PH + jh + 1],
                    lhsT=W[(wk, oh)][:, jh * 128 : (jh + 1) * 128],
                    rhs=B[("bq_" + br, oh)],
                    start=(oh == 0),
                    stop=(oh == PH - 1),
                )
        tU = consts.tile([128, PH], F32, tag=f"U{br}", name=f"U{br}")
        nc.vector.tensor_scalar_mul(out=tU, in0=ps[:, 0:PH], scalar1=float(sign * N))
        tR = consts.tile([128, PH], FP16, tag=f"R{br}", name=f"R{br}")
        nc.vector.tensor_scalar_mul(out=tR, in0=ps[:, PH : 2 * PH], scalar1=sign)
        U[("full", br)] = tU
        for ih in range(PH):
            U[(br, ih)] = tU[:, ih : ih + 1]
        for jh in range(PH):
            R[(br, jh)] = tR[:, jh : jh + 1]

    # batch-independent bias-dot part of c_diff: N*(bq_a.bk_a - bq_d.bk_d)
    ps = ps_sm.tile([1, 1], F32, tag="ps_sm", name="psCb")
    k = 0
    for bq, bk, sgn in (("bq_a", "bk_a", 1), ("bq_d", "bk_d", -1)):
        for oh in range(PH):
            t = consts.tile([128, 1], F32, tag=f"bkN{bk}{oh}", name=f"bkN{bk}{oh}")
            nc.vector.tensor_scalar_mul(out=t, in0=B[(bk, oh)], scalar1=float(sgn * N))
            nc.tensor.matmul(ps, lhsT=B[(bq, oh)], rhs=t, start=(k == 0), stop=(k == 3))
            k += 1
    c_bias = consts.tile([1, 1], FP16, tag="c_bias")
    nc.vector.tensor_scalar_mul(out=c_bias, in0=ps, scalar1=1.0)
    return PT, U, R, c_bias


def _body(tc, rgb, evt, wts, bss, out):
    nc = tc.nc
    ACT = mybir.ActivationFunctionType
    with ExitStack() as ctx:
        consts = ctx.enter_context(tc.tile_pool(name="consts", bufs=1))
        data = ctx.enter_context(tc.tile_pool(name="data", bufs=2))
        mpool = ctx.enter_context(tc.tile_pool(name="mpool", bufs=2))
        wbp = ctx.enter_context(tc.tile_pool(name="wbp", bufs=2))
        small = ctx.enter_context(tc.tile_pool(name="small", bufs=2))
        wchunk = ctx.enter_context(tc.tile_pool(name="wchunk", bufs=4))
        ps_val = ctx.enter_context(tc.tile_pool(name="ps_val", bufs=5, space="PSUM"))
        ps_wb = ctx.enter_context(tc.tile_pool(name="ps_wb", bufs=3, space="PSUM"))
        ps_sm = ctx.enter_context(tc.tile_pool(name="ps_sm", bufs=1, space="PSUM"))

        # ---- load weights + biases (plain f32 HWDGE) -------------------
        W = {}
        for nm in ("Wq_a", "Wk_a", "Wq_d", "Wk_d"):
            for h in range(PH):
                t = consts.tile([128, DIM], F32, tag=f"{nm}{h}", name=f"{nm}{h}")
                nc.sync.dma_start(out=t, in_=wts[nm][h])
                W[(nm, h)] = t
        B = {}
        for nm in ("bq_a", "bk_a", "bq_d", "bk_d"):
            for h in range(PH):
                t = consts.tile([128, 1], F32, tag=f"{nm}{h}", name=f"b{nm}{h}")
                nc.sync.dma_start(out=t, in_=bss[nm][h])
                B[(nm, h)] = t

        ones_row = consts.tile([1, 128], FP16, tag="ones")
        nc.vector.memset(ones_row, 1.0)
        one_one = consts.tile([1, 1], FP16, tag="one_one")
        nc.vector.memset(one_one, 1.0)
        garbage = consts.tile([128, 1], F32, tag="garbage")

        PT, U, R, c_bias = _precompute(tc, consts, ps_sm, W, B)

        st = [dict() for _ in range(BPC)]

        def emit_loads(b, after=None):
            # casting SWDGE loads: DRAM f32 -> SBUF fp16. `after` orders this
            # batch's stream behind the previous batch's last load so early
            # DMAs complete early (completions smear across co-resident DMAs).
            A, Dv = {}, {}
            first = last = None
            for h in range(PH):
                A[h] = data.tile([128, N], FP16, tag=f"A{h}", name=f"A{h}_{b}")
                Dv[h] = data.tile([128, N], FP16, tag=f"D{h}", name=f"D{h}_{b}")
                for blk in range(NLB):
                    sl = slice(blk * LB, (blk + 1) * LB)
                    i1 = nc.gpsimd.dma_start(out=A[h][:, sl], in_=rgb[b, h][:, sl])
                    i2 = nc.gpsimd.dma_start(out=Dv[h][:, sl], in_=evt[b, h][:, sl])
                    if first is None:
                        first = i1
                    last = i2
            if after is not None:
                _add_dep_helper(
                    first.ins, after.ins, sync=True,
                    reason="batch loads ordered to avoid completion smearing",
                )
            st[b] = dict(A=A, Dv=Dv, last_load=last)

        def make_red_ops(b):
            # a-branch row-sum partials on ScalarE (Copy + accum, discard out)
            A = st[b]["A"]
            SA = {}
            ops = []

            def red(t, s, dst):
                with nc.allow_low_precision(
                    reason="fp16 write of f32-accumulated partial"
                ):
                    nc.scalar.activation(
                        out=garbage.broadcast_to([128, RB]),
                        in_=t[:, s],
                        func=ACT.Copy,
                        accum_out=dst,
                    )

            for h in range(PH):
                s4 = small.tile([128, NRB], FP16, tag=f"s4a{h}", name=f"s4a{h}_{b}")
                SA[h] = s4
                for rb in range(NRB):
                    sl = slice(rb * RB, (rb + 1) * RB)
                    ops.append(lambda t=A[h], s=sl, dst=s4[:, rb : rb + 1]: red(t, s, dst))
            st[b]["SA"] = SA
            return ops

        def emit_subs(b):
            # M = A - D on DVE (fp16 2x), accumulating sm = rowsum(A - D)
            A, Dv = st[b]["A"], st[b]["Dv"]
            M, SM = {}, {}
            for h in range(PH):
                M[h] = mpool.tile([128, N], FP16, tag=f"M{h}", name=f"M{h}_{b}")
                sm4 = small.tile([128, NRB], F32, tag=f"sm4{h}", name=f"sm4{h}_{b}")
                SM[h] = sm4
                for rb in range(NRB):
                    sl = slice(rb * RB, (rb + 1) * RB)
                    nc.vector.scalar_tensor_tensor(
                        out=M[h][:, sl],
                        in0=A[h][:, sl],
                        scalar=1.0,
                        in1=Dv[h][:, sl],
                        op0=mybir.AluOpType.mult,
                        op1=mybir.AluOpType.subtract,
                        accum_out=sm4[:, rb : rb + 1],
                    )
            st[b]["M"] = M
            st[b]["SM"] = SM

        def stage2_head(b):
            A, Dv, M = st[b]["A"], st[b]["Dv"], st[b]["M"]
            SA, SM = st[b]["SA"], st[b]["SM"]

            # d-branch partials: s_d = s_a - sm (tiny)
            S4 = {}
            for h in range(PH):
                S4[("a", h)] = SA[h]
                sd = small.tile([128, NRB], FP16, tag=f"s4d{h}", name=f"s4d{h}_{b}")
                with nc.allow_low_precision(reason="tiny fp16 partial"):
                    nc.vector.tensor_sub(out=sd, in0=SA[h], in1=SM[h])
                S4[("d", h)] = sd

            # c_diff = sum_j r[j] s[j] (both branches) + c_bias
            ps_c = ps_sm.tile([1, 1], F32, tag="ps_sm", name=f"psc_{b}")
            terms = [
                (S4[(br, jh)][:, rb : rb + 1], R[(br, jh)])
                for br in ("a", "d")
                for jh in range(PH)
                for rb in range(NRB)
            ]
            for i, (l, r) in enumerate(terms):
                nc.tensor.matmul(ps_c, lhsT=l, rhs=r, start=(i == 0), stop=False)
            nc.tensor.matmul(ps_c, lhsT=c_bias, rhs=one_one, start=False, stop=True)
            c16 = small.tile([1, 1], F32, tag="c16", name=f"c16_{b}")
            nc.scalar.mul(out=c16, in_=ps_c, mul=INV_SQRT_D)

            # v = PT @ s + U per branch, split into fp16 hi+lo
            VH, VL = {}, {}
            for br in ("a", "d"):
                ps = ps_sm.tile([128, PH], F32, tag="ps_sm", name=f"psv{br}_{b}")
                for ih in range(PH):
                    k = 0
                    for jh in range(PH):
                        for rb in range(NRB):
                            nc.tensor.matmul(
                                ps[:, ih : ih + 1],
                                lhsT=PT[(br, jh)][:, ih * 128 : (ih + 1) * 128],
                                rhs=S4[(br, jh)][:, rb : rb + 1],
                                start=(k == 0),
                                stop=(k == PH * NRB - 1),
                            )
                            k += 1
                v = small.tile([128, PH], F32, tag=f"v{br}", name=f"v{br}_{b}")
                nc.vector.tensor_add(out=v, in0=ps, in1=U[("full", br)])
                vh = small.tile([128, PH], FP16, tag=f"vh{br}", name=f"vh{br}_{b}")
                nc.vector.tensor_scalar_mul(out=vh, in0=v, scalar1=1.0)
                vl = small.tile([128, PH], FP16, tag=f"vl{br}", name=f"vl{br}_{b}")
                nc.vector.tensor_sub(out=vl, in0=v, in1=vh)
                for ih in range(PH):
                    VH[(br, ih)] = vh[:, ih : ih + 1]
                    VL[(br, ih)] = vl[:, ih : ih + 1]
            st[b]["VH"], st[b]["VL"], st[b]["c16"] = VH, VL, c16

        def stage2_value(b, interleave_ops=()):
            A, Dv = st[b]["A"], st[b]["Dv"]
            VH, VL, c16 = st[b]["VH"], st[b]["VL"], st[b]["c16"]
            il = list(interleave_ops)
            # value chunks: 8 fp16 matmuls -> sigmoid -> partition_broadcast
            mms = [
                (VH[("a", 0)], A[0]), (VL[("a", 0)], A[0]),
                (VH[("a", 1)], A[1]), (VL[("a", 1)], A[1]),
                (VH[("d", 0)], Dv[0]), (VL[("d", 0)], Dv[0]),
                (VH[("d", 1)], Dv[1]), (VL[("d", 1)], Dv[1]),
            ]
            wb_sb = wbp.tile([128, N], FP16, tag="wb_sb", name=f"wb_sb_{b}")
            for ich in range(NCH):
                sl = slice(ich * CH, (ich + 1) * CH)
                psv = ps_val.tile([1, CH], F32, tag="psv", name=f"psval{ich}_{b}")
                for i, (v, t) in enumerate(mms):
                    nc.tensor.matmul(
                        psv, lhsT=v, rhs=t[:, sl],
                        start=(i == 0), stop=(i == len(mms) - 1),
                    )
                if il:
                    il.pop(0)()
                wrow = wchunk.tile([1, CH], FP16, tag="wrow", name=f"wrow{ich}_{b}")
                nc.scalar.activation(
                    out=wrow, in_=psv,
                    func=ACT.Sigmoid, bias=c16, scale=INV_SQRT_D,
                )
                nc.gpsimd.partition_broadcast(wb_sb[:, sl], wrow)
            for op in il:
                op()
            st[b]["wb_sb"] = wb_sb

        def stage2_blend(b):
            A, Dv, M, wb_sb = st[b]["A"], st[b]["Dv"], st[b]["M"], st[b]["wb_sb"]
            # blend: M *= wb (2x), A = M + D (in-place A), casting stores
            for h in range(PH):
                for sb in range(NSB):
                    sl = slice(sb * SB, (sb + 1) * SB)
                    nc.vector.tensor_mul(out=M[h][:, sl], in0=M[h][:, sl], in1=wb_sb[:, sl])
            for h in range(PH):
                for sb in range(NSB):
                    sl = slice(sb * SB, (sb + 1) * SB)
                    nc.vector.tensor_add(out=A[h][:, sl], in0=M[h][:, sl], in1=Dv[h][:, sl])
                    nc.gpsimd.dma_start(out=out[b, h][:, sl], in_=A[h][:, sl])

        # ---- emission schedule ----------------------------------------
        emit_loads(0)
        for op in make_red_ops(0):
            op()
        emit_subs(0)
        emit_loads(1, after=st[0]["last_load"])
        red1 = make_red_ops(1)
        stage2_head(0)
        stage2_value(0, interleave_ops=red1)
        emit_subs(1)
        stage2_head(1)
        stage2_value(1)
        stage2_blend(0)
        stage2_blend(1)


_NC_CACHE = None


def _get_nc():
    global _NC_CACHE
    if _NC_CACHE is None:
        _NC_CACHE = build_nc()
    return _NC_CACHE


def _make_in_maps(inputs):
    rgb = np.ascontiguousarray(np.asarray(inputs["rgb"], dtype=np.float32)).reshape(
        BS, PH, 128, N
    )
    evt = np.ascontiguousarray(np.asarray(inputs["evt"], dtype=np.float32)).reshape(
        BS, PH, 128, N
    )
    base = {}
    for nm in ("Wq_a", "Wk_a", "Wq_d", "Wk_d"):
        base[nm] = np.ascontiguousarray(
            np.asarray(inputs[nm], dtype=np.float32)
        ).reshape(PH, 128, DIM)
    for nm in ("bq_a", "bk_a", "bq_d", "bk_d"):
        base[nm] = np.ascontiguousarray(
            np.asarray(inputs[nm], dtype=np.float32)
        ).reshape(PH, 128, 1)
    in_maps = []
    for c in range(NCORES):
        m = dict(base)
        m["rgb"] = np.ascontiguousarray(rgb[c * BPC : (c + 1) * BPC])
        m["evt"] = np.ascontiguousarray(evt[c * BPC : (c + 1) * BPC])
        in_maps.append(m)
    return in_maps


def run(inputs, trace=False):
    nc = _get_nc()
    in_maps = _make_in_maps(inputs)
    res = run_bass_kernel_spmd(nc, in_maps, core_ids=list(range(NCORES)), trace=trace)
    outs = [
        np.asarray(res.results[i]["out"]).reshape(BPC, DIM, HH, WW)
        for i in range(NCORES)
    ]
    full = np.concatenate(outs, axis=0)
    return full, res


def kernel(**inputs) -> np.ndarray:
    full, _ = run(inputs, trace=False)
    return full


# revision 21
# speedup vs baseline: 1.3425x; 1.0092x over previous
"""Trainium2 Bass kernel for the two-branch sparse-attention fusion module.

Math (per batch b, tokens T = rgb/evt as (d=256, N=4096) d-major):
    s      = sum_n T[:, n]                           (256,)
    value[n] = T[:,n].v + c, v = (Wk^T Wq)^T s + N Wq^T bk, c = (Wk^T bq).s + N bq.bk
    w      = sigmoid((value_rgb - value_evt)/sqrt(d))
    out    = evt + w * (rgb - evt)

All-fp16 dataflow (fp32 only for DRAM I/O and PSUM accumulation):
    DMA (SWDGE): casting loads f32->fp16, casting stores fp16->f32
    ScalarE    : streaming row-sum partials (Copy+accum_out), sigmoid,
                 wb PSUM->SBUF fp16 copies, v_hi casts
    PE         : weight-product matvecs, fp16 value matmuls with v split
                 into hi+lo fp16 halves (error compensation), K=1 fp16
                 broadcast of w to 128 partitions
    DVE        : in-place fp16 blend A=(A-D), A*=wb, A+=D (2x mode)

Sharded data-parallel over batch: 8 cores x 2 batches, weights replicated.
"""

import numpy as np
from contextlib import ExitStack

import concourse.bass as bass
import concourse.tile as tile
from concourse import bacc, mybir
from concourse.bass import _add_dep_helper
from concourse.bass_utils import run_bass_kernel_spmd

F32 = mybir.dt.float32
FP16 = mybir.dt.float16

BS, DIM, HH, WW = 16, 256, 64, 64
N = HH * WW                 # 4096 tokens
NCORES = 8
BPC = BS // NCORES          # batches per core
PH = DIM // 128             # partition halves of the d dim
CH = 512                    # value-chunk (one PSUM bank of f32)
NCH = N // CH               # 8
LB = 2048                   # load block columns (1 MiB DRAM-side)
NLB = N // LB               # 2
RB = 2048                   # reduce block columns
NRB = N // RB               # 2
SB = 2048                   # store/blend block columns
NSB = N // SB               # 2
INV_SQRT_D = 1.0 / 16.0


def build_nc() -> bass.Bass:
    nc = bacc.Bacc()

    rgb = nc.declare_dram_parameter("rgb", [BPC, PH, 128, N], F32, isOutput=False)
    evt = nc.declare_dram_parameter("evt", [BPC, PH, 128, N], F32, isOutput=False)
    wts = {}
    for nm in ("Wq_a", "Wk_a", "Wq_d", "Wk_d"):
        wts[nm] = nc.declare_dram_parameter(nm, [PH, 128, DIM], F32, isOutput=False)
    bss = {}
    for nm in ("bq_a", "bk_a", "bq_d", "bk_d"):
        bss[nm] = nc.declare_dram_parameter(nm, [PH, 128, 1], F32, isOutput=False)
    out = nc.declare_dram_parameter("out", [BPC, PH, 128, N], F32, isOutput=True)

    with tile.TileContext(nc) as tc:
        _body(tc, rgb, evt, wts, bss, out)
    nc.finalize()
    return nc


def _precompute(tc, consts, ps_sm, W, B):
    """Weight products; the d branch carries a folded minus sign.
    PT and R are stored fp16 (they feed fp16 matvecs); U stays f32."""
    nc = tc.nc
    PT, U, R = {}, {}, {}
    for br, wq, wk, sign in (
        ("a", "Wq_a", "Wk_a", 1.0),
        ("d", "Wq_d", "Wk_d", -1.0),
    ):
        for jh in range(PH):
            ps = ps_sm.tile([128, DIM], F32, tag="ps_sm", name=f"psPT{br}{jh}")
            for oh in range(PH):
                nc.tensor.matmul(
                    ps,
                    lhsT=W[(wk, oh)][:, jh * 128 : (jh + 1) * 128],
                    rhs=W[(wq, oh)],
                    start=(oh == 0),
                    stop=(oh == PH - 1),
                )
            t = consts.tile([128, DIM], FP16, tag=f"PT{br}{jh}", name=f"PT{br}{jh}")
            nc.vector.tensor_scalar_mul(out=t, in0=ps, scalar1=sign)
            PT[(br, jh)] = t
        ps = ps_sm.tile([128, 2 * PH], F32, tag="ps_sm", name=f"psUR{br}")
        for ih in range(PH):
            for oh in range(PH):
                nc.tensor.matmul(
                    ps[:, ih : ih + 1],
                    lhsT=W[(wq, oh)][:, ih * 128 : (ih + 1) * 128],
                    rhs=B[("bk_" + br, oh)],
                    start=(oh == 0),
                    stop=(oh == PH - 1),
                )
        for jh in range(PH):
            for oh in range(PH):
                nc.tensor.matmul(
                    ps[:, PH + jh : PH + jh + 1],
                    lhsT=W[(wk, oh)][:, jh * 128 : (jh + 1) * 128],
                    rhs=B[("bq_" + br, oh)],
                    start=(oh == 0),
                    stop=(oh == PH - 1),
                )
        tU = consts.tile([128, PH], F32, tag=f"U{br}", name=f"U{br}")
        nc.vector.tensor_scalar_mul(out=tU, in0=ps[:, 0:PH], scalar1=float(sign * N))
        tR = consts.tile([128, PH], FP16, tag=f"R{br}", name=f"R{br}")
        nc.vector.tensor_scalar_mul(out=tR, in0=ps[:, PH : 2 * PH], scalar1=sign)
        U[("full", br)] = tU
        for ih in range(PH):
            U[(br, ih)] = tU[:, ih : ih + 1]
        for jh in range(PH):
            R[(br, jh)] = tR[:, jh : jh + 1]

    # batch-independent bias-dot part of c_diff: N*(bq_a.bk_a - bq_d.bk_d)
    ps = ps_sm.tile([1, 1], F32, tag="ps_sm", name="psCb")
    k = 0
    for bq, bk, sgn in (("bq_a", "bk_a", 1), ("bq_d", "bk_d", -1)):
        for oh in range(PH):
            t = consts.tile([128, 1], F32, tag=f"bkN{bk}{oh}", name=f"bkN{bk}{oh}")
            nc.vector.tensor_scalar_mul(out=t, in0=B[(bk, oh)], scalar1=float(sgn * N))
            nc.tensor.matmul(ps, lhsT=B[(bq, oh)], rhs=t, start=(k == 0), stop=(k == 3))
            k += 1
    c_bias = consts.tile([1, 1], FP16, tag="c_bias")
    nc.vector.tensor_scalar_mul(out=c_bias, in0=ps, scalar1=1.0)
    return PT, U, R, c_bias


def _body(tc, rgb, evt, wts, bss, out):
    nc = tc.nc
    ACT = mybir.ActivationFunctionType
    with ExitStack() as ctx:
        consts = ctx.enter_context(tc.tile_pool(name="consts", bufs=1))
        data = ctx.enter_context(tc.tile_pool(name="data", bufs=2))
        mpool = ctx.enter_context(tc.tile_pool(name="mpool", bufs=2))
        wbp = ctx.enter_context(tc.tile_pool(name="wbp", bufs=2))
        small = ctx.enter_context(tc.tile_pool(name="small", bufs=2))
        wchunk = ctx.enter_context(tc.tile_pool(name="wchunk", bufs=4))
        ps_val = ctx.enter_context(tc.tile_pool(name="ps_val", bufs=5, space="PSUM"))
        ps_wb = ctx.enter_context(tc.tile_pool(name="ps_wb", bufs=3, space="PSUM"))
        ps_sm = ctx.enter_context(tc.tile_pool(name="ps_sm", bufs=1, space="PSUM"))

        # ---- load weights + biases (one SWDGE DMA per tensor, ahead of
        # the token stream so the precompute never starves) ---------------
        W = {}
        for nm in ("Wq_a", "Wk_a", "Wq_d", "Wk_d"):
            t = consts.tile([128, PH * DIM], F32, tag=nm, name=f"W{nm}")
            nc.gpsimd.dma_start(
                out=t.rearrange("p (h c) -> p h c", h=PH),
                in_=wts[nm].rearrange("h p c -> p h c"),
            )
            for h in range(PH):
                W[(nm, h)] = t[:, h * DIM : (h + 1) * DIM]
        B = {}
        for nm in ("bq_a", "bk_a", "bq_d", "bk_d"):
            t = consts.tile([128, PH], F32, tag=f"b{nm}", name=f"b{nm}")
            nc.gpsimd.dma_start(
                out=t.rearrange("p (h c) -> p h c", h=PH),
                in_=bss[nm].rearrange("h p c -> p h c"),
            )
            for h in range(PH):
                B[(nm, h)] = t[:, h : h + 1]

        ones_row = consts.tile([1, 128], FP16, tag="ones")
        nc.vector.memset(ones_row, 1.0)
        one_one = consts.tile([1, 1], FP16, tag="one_one")
        nc.vector.memset(one_one, 1.0)
        garbage = consts.tile([128, 1], F32, tag="garbage")

        PT, U, R, c_bias = _precompute(tc, consts, ps_sm, W, B)

        st = [dict() for _ in range(BPC)]

        def emit_loads(b, after=None):
            # casting SWDGE loads: DRAM f32 -> SBUF fp16. `after` orders this
            # batch's stream behind the previous batch's last load so early
            # DMAs complete early (completions smear across co-resident DMAs).
            A, Dv = {}, {}
            first = last = None
            for h in range(PH):
                A[h] = data.tile([128, N], FP16, tag=f"A{h}", name=f"A{h}_{b}")
                Dv[h] = data.tile([128, N], FP16, tag=f"D{h}", name=f"D{h}_{b}")
                for blk in range(NLB):
                    sl = slice(blk * LB, (blk + 1) * LB)
                    i1 = nc.gpsimd.dma_start(out=A[h][:, sl], in_=rgb[b, h][:, sl])
                    i2 = nc.gpsimd.dma_start(out=Dv[h][:, sl], in_=evt[b, h][:, sl])
                    if first is None:
                        first = i1
                    last = i2
            if after is not None:
                _add_dep_helper(
                    first.ins, after.ins, sync=True,
                    reason="batch loads ordered to avoid completion smearing",
                )
            st[b] = dict(A=A, Dv=Dv, last_load=last)

        def make_red_ops(b):
            # a-branch row-sum partials on ScalarE (Copy + accum, discard out)
            A = st[b]["A"]
            SA = {}
            ops = []

            def red(t, s, dst):
                with nc.allow_low_precision(
                    reason="fp16 write of f32-accumulated partial"
                ):
                    nc.scalar.activation(
                        out=garbage.broadcast_to([128, RB]),
                        in_=t[:, s],
                        func=ACT.Copy,
                        accum_out=dst,
                    )

            for h in range(PH):
                s4 = small.tile([128, NRB], FP16, tag=f"s4a{h}", name=f"s4a{h}_{b}")
                SA[h] = s4
                for rb in range(NRB):
                    sl = slice(rb * RB, (rb + 1) * RB)
                    ops.append(lambda t=A[h], s=sl, dst=s4[:, rb : rb + 1]: red(t, s, dst))
            st[b]["SA"] = SA
            return ops

        def make_sub_ops(b):
            # M = A - D on DVE (fp16 2x), accumulating sm = rowsum(A - D)
            A, Dv = st[b]["A"], st[b]["Dv"]
            M, SM = {}, {}
            ops = []

            def sub(mh, ah, dh, s, dst):
                nc.vector.scalar_tensor_tensor(
                    out=mh[:, s],
                    in0=ah[:, s],
                    scalar=1.0,
                    in1=dh[:, s],
                    op0=mybir.AluOpType.mult,
                    op1=mybir.AluOpType.subtract,
                    accum_out=dst,
                )

            for h in range(PH):
                M[h] = mpool.tile([128, N], FP16, tag=f"M{h}", name=f"M{h}_{b}")
                sm4 = small.tile([128, NRB], F32, tag=f"sm4{h}", name=f"sm4{h}_{b}")
                SM[h] = sm4
                for rb in range(NRB):
                    sl = slice(rb * RB, (rb + 1) * RB)
                    ops.append(
                        lambda mh=M[h], ah=A[h], dh=Dv[h], s=sl,
                        dst=sm4[:, rb : rb + 1]: sub(mh, ah, dh, s, dst)
                    )
            st[b]["M"] = M
            st[b]["SM"] = SM
            return ops

        def stage2_head(b):
            A, Dv, M = st[b]["A"], st[b]["Dv"], st[b]["M"]
            SA, SM = st[b]["SA"], st[b]["SM"]

            # d-branch partials: s_d = s_a - sm (tiny)
            S4 = {}
            for h in range(PH):
                S4[("a", h)] = SA[h]
                sd = small.tile([128, NRB], FP16, tag=f"s4d{h}", name=f"s4d{h}_{b}")
                with nc.allow_low_precision(reason="tiny fp16 partial"):
                    nc.vector.tensor_sub(out=sd, in0=SA[h], in1=SM[h])
                S4[("d", h)] = sd

            # c_diff = sum_j r[j] s[j] (both branches) + c_bias
            ps_c = ps_sm.tile([1, 1], F32, tag="ps_sm", name=f"psc_{b}")
            terms = [
                (S4[(br, jh)][:, rb : rb + 1], R[(br, jh)])
                for br in ("a", "d")
                for jh in range(PH)
                for rb in range(NRB)
            ]
            for i, (l, r) in enumerate(terms):
                nc.tensor.matmul(ps_c, lhsT=l, rhs=r, start=(i == 0), stop=False)
            nc.tensor.matmul(ps_c, lhsT=c_bias, rhs=one_one, start=False, stop=True)
            c16 = small.tile([1, 1], F32, tag="c16", name=f"c16_{b}")
            nc.scalar.mul(out=c16, in_=ps_c, mul=INV_SQRT_D)

            # v = PT @ s + U per branch, split into fp16 hi+lo
            VH, VL = {}, {}
            for br in ("a", "d"):
                ps = ps_sm.tile([128, PH], F32, tag="ps_sm", name=f"psv{br}_{b}")
                for ih in range(PH):
                    k = 0
                    for jh in range(PH):
                        for rb in range(NRB):
                            nc.tensor.matmul(
                                ps[:, ih : ih + 1],
                                lhsT=PT[(br, jh)][:, ih * 128 : (ih + 1) * 128],
                                rhs=S4[(br, jh)][:, rb : rb + 1],
                                start=(k == 0),
                                stop=(k == PH * NRB - 1),
                            )
                            k += 1
                v = small.tile([128, PH], F32, tag=f"v{br}", name=f"v{br}_{b}")
                nc.vector.tensor_add(out=v, in0=ps, in1=U[("full", br)])
                vh = small.tile([128, PH], FP16, tag=f"vh{br}", name=f"vh{br}_{b}")
                nc.vector.tensor_scalar_mul(out=vh, in0=v, scalar1=1.0)
                vl = small.tile([128, PH], FP16, tag=f"vl{br}", name=f"vl{br}_{b}")
                nc.vector.tensor_sub(out=vl, in0=v, in1=vh)
                for ih in range(PH):
                    VH[(br, ih)] = vh[:, ih : ih + 1]
                    VL[(br, ih)] = vl[:, ih : ih + 1]
            st[b]["VH"], st[b]["VL"], st[b]["c16"] = VH, VL, c16

        def stage2_value(b, interleave_ops=()):
            A, Dv = st[b]["A"], st[b]["Dv"]
            VH, VL, c16 = st[b]["VH"], st[b]["VL"], st[b]["c16"]
            il = list(interleave_ops)
            # value chunks: 8 fp16 matmuls -> sigmoid -> partition_broadcast
            mms = [
                (VH[("a", 0)], A[0]), (VL[("a", 0)], A[0]),
                (VH[("a", 1)], A[1]), (VL[("a", 1)], A[1]),
                (VH[("d", 0)], Dv[0]), (VL[("d", 0)], Dv[0]),
                (VH[("d", 1)], Dv[1]), (VL[("d", 1)], Dv[1]),
            ]
            wb_sb = wbp.tile([128, N], FP16, tag="wb_sb", name=f"wb_sb_{b}")
            for ich in range(NCH):
                sl = slice(ich * CH, (ich + 1) * CH)
                psv = ps_val.tile([1, CH], F32, tag="psv", name=f"psval{ich}_{b}")
                for i, (v, t) in enumerate(mms):
                    nc.tensor.matmul(
                        psv, lhsT=v, rhs=t[:, sl],
                        start=(i == 0), stop=(i == len(mms) - 1),
                    )
                if il:
                    il.pop(0)()
                wrow = wchunk.tile([1, CH], FP16, tag="wrow", name=f"wrow{ich}_{b}")
                nc.scalar.activation(
                    out=wrow, in_=psv,
                    func=ACT.Sigmoid, bias=c16, scale=INV_SQRT_D,
                )
                nc.gpsimd.partition_broadcast(wb_sb[:, sl], wrow)
            for op in il:
                op()
            st[b]["wb_sb"] = wb_sb

        def stage2_blend(b, interleave_ops=()):
            A, Dv, M, wb_sb = st[b]["A"], st[b]["Dv"], st[b]["M"], st[b]["wb_sb"]
            il = list(interleave_ops)
            # blend: M *= wb (2x), A = M + D (in-place A), casting stores
            for h in range(PH):
                for sb in range(NSB):
                    sl = slice(sb * SB, (sb + 1) * SB)
                    if il:
                        il.pop(0)()
                    nc.vector.tensor_mul(out=M[h][:, sl], in0=M[h][:, sl], in1=wb_sb[:, sl])
            for h in range(PH):
                for sb in range(NSB):
                    sl = slice(sb * SB, (sb + 1) * SB)
                    if il:
                        il.pop(0)()
                    nc.vector.tensor_add(out=A[h][:, sl], in0=M[h][:, sl], in1=Dv[h][:, sl])
                    nc.gpsimd.dma_start(out=out[b, h][:, sl], in_=A[h][:, sl])
            for op in il:
                op()

        # ---- emission schedule ----------------------------------------
        emit_loads(0)
        for op in make_red_ops(0):
            op()
        for op in make_sub_ops(0):
            op()
        emit_loads(1, after=st[0]["last_load"])
        red1 = make_red_ops(1)
        sub1 = make_sub_ops(1)
        stage2_head(0)
        stage2_value(0, interleave_ops=red1)
        stage2_blend(0, interleave_ops=sub1)
        stage2_head(1)
        stage2_value(1)
        stage2_blend(1)


_NC_CACHE = None


def _get_nc():
    global _NC_CACHE
    if _NC_CACHE is None:
        _NC_CACHE = build_nc()
    return _NC_CACHE


def _make_in_maps(inputs):
    rgb = np.ascontiguousarray(np.asarray(inputs["rgb"], dtype=np.float32)).reshape(
        BS, PH, 128, N
    )
    evt = np.ascontiguousarray(np.asarray(inputs["evt"], dtype=np.float32)).reshape(
        BS, PH, 128, N
    )
    base = {}
    for nm in ("Wq_a", "Wk_a", "Wq_d", "Wk_d"):
        base[nm] = np.ascontiguousarray(
            np.asarray(inputs[nm], dtype=np.float32)
        ).reshape(PH, 128, DIM)
    for nm in ("bq_a", "bk_a", "bq_d", "bk_d"):
        base[nm] = np.ascontiguousarray(
            np.asarray(inputs[nm], dtype=np.float32)
        ).reshape(PH, 128, 1)
    in_maps = []
    for c in range(NCORES):
        m = dict(base)
        m["rgb"] = np.ascontiguousarray(rgb[c * BPC : (c + 1) * BPC])
        m["evt"] = np.ascontiguousarray(evt[c * BPC : (c + 1) * BPC])
        in_maps.append(m)
    return in_maps


def run(inputs, trace=False):
    nc = _get_nc()
    in_maps = _make_in_maps(inputs)
    res = run_bass_kernel_spmd(nc, in_maps, core_ids=list(range(NCORES)), trace=trace)
    outs = [
        np.asarray(res.results[i]["out"]).reshape(BPC, DIM, HH, WW)
        for i in range(NCORES)
    ]
    full = np.concatenate(outs, axis=0)
    return full, res


def kernel(**inputs) -> np.ndarray:
    full, _ = run(inputs, trace=False)
    return full
